# revision 11
# baseline (speedup 1.0000x reference)
"""ChildSumTreeLSTM with relation transforms on 8 Trainium2 NeuronCores.

Layout: transposed (features on SBUF partitions, tree nodes on the free dim),
feature-sharded state (each core owns a 128-feature slice of h/c/xi/gates).
Column order: wave 0 (leaves) in heap order; each internal wave's parent
columns are grouped by the core that owns their relation (8 equal padded
groups) so every cross-core exchange is an AllGather of the core's own
contiguous shard — no AllReduce anywhere:

  per wave: gather child cols -> hsum -> per-slot (rel) masked-rhs matmuls
  accumulated into one PSUM region (mask selects that rel's columns; the
  per-core psum is then nonzero only on the core's own column group) ->
  group-axis reduce packs the AG shard -> AllGather ch_sum -> column-sharded
  iou gates + f gates -> AllGather of the new h feature-slices.

The wave-4 single node's relation matrix is replicated on all cores (skips
its ch_sum exchange); the root uses the identity relation (no matmul).
A dummy warmup collective at program start absorbs the first-collective
mesh setup latency under the xi/xf matmuls.
All per-core differences are input data (weight slots, masks, bias slices),
so one Bass program runs SPMD on all 8 cores.
"""

import sys

sys.path.insert(0, "/opt/trn_rl_repo")

import numpy as np
import ml_dtypes

import concourse.bass as bass
import concourse.mybir as mybir
import concourse.tile as tile
from concourse.bass_utils import run_bass_kernel_spmd
from concourse.vector_clock import ScopedClock, VectorClock

BF16 = mybir.dt.bfloat16
F32 = mybir.dt.float32
NCORES = 8
P = 128


# This walrus build rejects >1 sem wait per instruction at the Tile exit
# drain; split the aggregated drain into one drain per proc.
def _split_drain_and_barrier(self, tick_clock, wait_clock):
    gc = tick_clock.global_clock
    n = len(gc)
    nonzero = [i for i in range(n) if gc[i] > 0]
    for j in nonzero:
        vec = VectorClock([gc[i] if i == j else 0 for i in range(n)])
        d = self.nc.sync.drain()
        wait_clock.add_sem_waits(d.ins, ScopedClock({None: vec}))
    if not nonzero:
        d = self.nc.sync.drain()
        wait_clock.add_sem_waits(d.ins, ScopedClock({None: gc.copy()}))
    self.nc.all_engine_barrier()
    assert self.sems is not None
    popped = self.nc._tile_sem_poison_stack.pop()
    assert popped is self._sem_poison
    self.nc.clear_and_free_semaphores(list(self.sems.allocated().values()))
    self.nc.all_engine_barrier()


tile.TileContext._drain_and_barrier = _split_drain_and_barrier


def _split_multi_waits(nc, limit=1):
    """Walrus here allows only one sem wait per instruction; hoist extras
    onto same-engine NOPs inserted right before the instruction."""
    for bb in nc.main_func.blocks:
        new_list = []
        for ins in bb.instructions:
            si = getattr(ins, "sync_info", None)
            if si is not None and si.on_wait and len(si.on_wait) > limit:
                waits = list(si.on_wait)
                for w in waits[:-limit]:
                    nop = mybir.InstNoOp(
                        name=nc.get_next_instruction_name(),
                        sync_info=mybir.SyncInfo(on_wait=[w], on_update=[]),
                        bass_nofuse=True,
                        engine=ins.engine,
                    )
                    nc.register_instruction(nop, overwrite=True)
                    new_list.append(nop)
                si.on_wait = waits[-limit:]
            new_list.append(ins)
        bb.instructions[:] = new_list


def _bf16(a):
    return np.ascontiguousarray(a.astype(ml_dtypes.bfloat16))


def _blocksT(mat):
    """[M, K] -> [K/128 * M/128, 128, 128] of transposed blocks, grouped as
    [m, k] -> index m*KC + k, each block = mat[mb, kb].T (lhsT)."""
    M, K = mat.shape
    MC, KC = M // P, K // P
    out = np.empty((MC * KC, P, P), mat.dtype)
    for m in range(MC):
        for k in range(KC):
            out[m * KC + k] = mat[m * P:(m + 1) * P, k * P:(k + 1) * P].T
    return out


def _runs(seq, zcol):
    """Maximal +1-contiguous runs of seq, skipping zcol entries.
    Returns list of (dst_off, src_col, length)."""
    runs = []
    i0 = 0
    n = len(seq)
    while i0 < n:
        if seq[i0] == zcol:
            i0 += 1
            continue
        i1 = i0 + 1
        while i1 < n and seq[i1] == seq[i1 - 1] + 1 and seq[i1] != zcol:
            i1 += 1
        runs.append((i0, int(seq[i0]), i1 - i0))
        i0 = i1
    return runs


def _plan(child_idx, rel_ids, Wrel):
    """Host-side planning: waves, rel->core assignment, grouped column
    order, per-wave child-gather runs."""
    N, K = child_idx.shape
    R1 = Wrel.shape[0]
    eff_children = []
    wave = np.zeros(N, np.int32)
    for i in range(N):
        cs = [int(c) for c in child_idx[i] if 0 <= c < i]
        eff_children.append(cs)
        wave[i] = 1 + max((wave[c] for c in cs), default=-1)
    nwaves = int(wave.max()) + 1

    ident = set()
    eye = np.eye(Wrel.shape[1], dtype=Wrel.dtype)
    for r in set(int(rel_ids[i]) for i in range(N)):
        if np.array_equal(Wrel[r], eye):
            ident.add(r)

    wave_nodes = [sorted([i for i in range(N) if wave[i] == w], key=lambda i: -i)
                  for w in range(nwaves)]

    # wave kinds: 0 = leaves; 'shard' = sharded rel + AGcs; 'repl' =
    # replicated weights (tiny waves); 'ident' = identity rel only
    kinds = []
    for w in range(1, nwaves):
        nodes = wave_nodes[w]
        rels = set(int(rel_ids[i]) for i in nodes)
        if rels <= ident:
            kinds.append("ident")
        elif len(nodes) == 1:
            kinds.append("repl")
        else:
            kinds.append("shard")

    # per sharded wave: assign rels -> cores (balance #rels, then #cols)
    wave_info = []
    for w in range(1, nwaves):
        nodes = wave_nodes[w]
        kind = kinds[w - 1]
        info = dict(kind=kind, nodes=nodes)
        if kind == "shard":
            from collections import Counter
            cnt = Counter(int(rel_ids[i]) for i in nodes)
            rels = sorted(cnt, key=lambda r: -cnt[r])
            ns = (len(rels) + NCORES - 1) // NCORES
            core_rels = [[] for _ in range(NCORES)]
            core_cols = [0] * NCORES
            for r in rels:
                best = min(range(NCORES),
                           key=lambda c: (len(core_rels[c]) >= ns,
                                          core_cols[c], len(core_rels[c])))
                core_rels[best].append(r)
                core_cols[best] += cnt[r]
            nmax = max(core_cols)
            # grouped node order: per core, by (rel, -node); pad to nmax
            grouped = []  # per col: node or None
            for c in range(NCORES):
                cn = [i for i in nodes if int(rel_ids[i]) in core_rels[c]]
                cn.sort(key=lambda i: (int(rel_ids[i]), -i))
                grouped.extend(cn)
                grouped.extend([None] * (nmax - len(cn)))
            info.update(ns=ns, core_rels=core_rels, nmax=nmax,
                        grouped=grouped, ncols=NCORES * nmax)
        else:
            info.update(grouped=list(nodes), ncols=len(nodes))
        wave_info.append(info)

    # global column order
    col_of = np.full(N, -1, np.int64)
    order_cols = []  # per col: node or None
    for i in wave_nodes[0]:
        col_of[i] = len(order_cols)
        order_cols.append(i)
    bases = [0]
    for info in wave_info:
        info["base"] = len(order_cols)
        bases.append(info["base"])
        for node in info["grouped"]:
            if node is not None:
                col_of[node] = len(order_cols)
            order_cols.append(node)
    C = len(order_cols)
    ZCOL = C
    NPAD = C + 6

    # child gather runs per wave (over grouped parent order; h and c share)
    for info in wave_info:
        seq = []
        for node in info["grouped"]:
            if node is None:
                seq.extend([ZCOL] * K)
            else:
                cs = eff_children[node]
                seq.extend([int(col_of[c]) for c in cs])
                seq.extend([ZCOL] * (K - len(cs)))
        info["runs"] = _runs(seq, ZCOL)
        info["has_missing"] = any(s == ZCOL for s in seq)

    return dict(wave=wave, nwaves=nwaves, wave_nodes=wave_nodes,
                wave_info=wave_info, col_of=col_of, order_cols=order_cols,
                C=C, ZCOL=ZCOL, NPAD=NPAD, ident=ident,
                eff_children=eff_children)


def kernel(**inputs):
    x = np.asarray(inputs["x"], np.float32)
    Wrel = np.asarray(inputs["Wrel"], np.float32)
    ioux_w = np.asarray(inputs["ioux_w"], np.float32)
    ioux_b = np.asarray(inputs["ioux_b"], np.float32)
    iouh_w = np.asarray(inputs["iouh_w"], np.float32)
    iouh_b = np.asarray(inputs["iouh_b"], np.float32)
    fx_w = np.asarray(inputs["fx_w"], np.float32)
    fx_b = np.asarray(inputs["fx_b"], np.float32)
    fh_w = np.asarray(inputs["fh_w"], np.float32)
    fh_b = np.asarray(inputs["fh_b"], np.float32)
    child_idx = np.asarray(inputs["child_idx"], np.int32)
    rel_ids = np.asarray(inputs["rel_ids"], np.int32)

    N, IN_DIM = x.shape
    MEM = fh_w.shape[0]
    KC = MEM // P            # 8 feature chunks
    KX = IN_DIM // P         # 8 input chunks
    K = child_idx.shape[1]   # max children (4)

    plan = _plan(child_idx, rel_ids, Wrel)
    wave_info, col_of = plan["wave_info"], plan["col_of"]
    order_cols, C, ZCOL, NPAD = plan["order_cols"], plan["C"], plan["ZCOL"], plan["NPAD"]
    n0 = len(plan["wave_nodes"][0])

    # ---- per-core host data -------------------------------------------------
    xT = np.zeros((IN_DIM, C), np.float32)
    for j, node in enumerate(order_cols):
        if node is not None:
            xT[:, j] = x[node]
    xT_b = np.zeros((KX, P, C), ml_dtypes.bfloat16)
    for k in range(KX):
        xT_b[k] = _bf16(xT[k * P:(k + 1) * P])

    # weight slots + masks
    shard_waves = [i for i in wave_info if i["kind"] == "shard"]
    repl_waves = [i for i in wave_info if i["kind"] == "repl"]
    S_total = sum(i["ns"] for i in shard_waves) + len(repl_waves)
    MTK = sum(i["ns"] * KC * i["ncols"] for i in shard_waves)

    wstream = [np.zeros((S_total, P, KC * KC, P), ml_dtypes.bfloat16)
               for _ in range(NCORES)]
    maskbuf = [np.zeros((P, max(MTK, 1)), ml_dtypes.bfloat16)
               for _ in range(NCORES)]
    soff = 0
    moff = 0
    for info in wave_info:
        if info["kind"] == "shard":
            base, nmax, ncols = info["base"], info["nmax"], info["ncols"]
            info["soff"], info["moff"] = soff, moff
            for c in range(NCORES):
                for s, r in enumerate(info["core_rels"][c]):
                    wstream[c][soff + s] = _blocksT(Wrel[r]).transpose(1, 0, 2)
                    m = np.zeros((KC, ncols), np.float32)
                    for t in range(ncols):
                        node = info["grouped"][t]
                        if node is not None and int(rel_ids[node]) == r:
                            m[:, t] = 1.0
                    mo = moff + s * KC * ncols
                    maskbuf[c][:, mo:mo + KC * ncols] = _bf16(
                        np.broadcast_to(m.reshape(1, -1), (P, KC * ncols)))
            soff += info["ns"]
            moff += info["ns"] * KC * ncols
        elif info["kind"] == "repl":
            info["soff"] = soff
            r = int(rel_ids[info["nodes"][0]])
            wb = _blocksT(Wrel[r]).transpose(1, 0, 2)
            for c in range(NCORES):
                wstream[c][soff] = wb
            soff += 1

    iouxstat = [np.zeros((KX * 3, P, P), ml_dtypes.bfloat16) for _ in range(NCORES)]
    iouhstat = [np.zeros((KC * 3, P, P), ml_dtypes.bfloat16) for _ in range(NCORES)]
    fxstat = [np.zeros((KX, P, P), ml_dtypes.bfloat16) for _ in range(NCORES)]
    fhstat = [np.zeros((KC, P, P), ml_dtypes.bfloat16) for _ in range(NCORES)]
    b_xi = [np.zeros((3, P), np.float32) for _ in range(NCORES)]
    b_iou = [np.zeros((3, P), np.float32) for _ in range(NCORES)]
    b_xf = [np.zeros((P,), np.float32) for _ in range(NCORES)]
    b_fh = [np.zeros((P,), np.float32) for _ in range(NCORES)]
    for c in range(NCORES):
        rows = slice(c * P, (c + 1) * P)
        for g in range(3):
            gr = slice(g * MEM + c * P, g * MEM + (c + 1) * P)
            b_xi[c][g] = ioux_b[gr]
            b_iou[c][g] = iouh_b[gr]
            for k in range(KX):
                iouxstat[c][k * 3 + g] = _bf16(ioux_w[gr, k * P:(k + 1) * P].T)
            for k in range(KC):
                iouhstat[c][k * 3 + g] = _bf16(iouh_w[gr, k * P:(k + 1) * P].T)
        b_xf[c] = fx_b[rows]
        b_fh[c] = fh_b[rows]
        for k in range(KX):
            fxstat[c][k] = _bf16(fx_w[rows, k * P:(k + 1) * P].T)
        for k in range(KC):
            fhstat[c][k] = _bf16(fh_w[rows, k * P:(k + 1) * P].T)

    # ---- build program ------------------------------------------------------
    nc = bass.Bass("TRN2", target_bir_lowering=False, debug=False,
                   num_devices=NCORES)
    d_ws = nc.dram_tensor("wstream", list(wstream[0].shape), BF16,
                          kind="ExternalInput")
    d_mask = nc.dram_tensor("masks", list(maskbuf[0].shape), BF16,
                            kind="ExternalInput")
    d_xt = nc.dram_tensor("xt", [KX, P, C], BF16, kind="ExternalInput")
    d_iouxs = nc.dram_tensor("iouxstat", [KX * 3, P, P], BF16, kind="ExternalInput")
    d_iouhs = nc.dram_tensor("iouhstat", [KC * 3, P, P], BF16, kind="ExternalInput")
    d_fxs = nc.dram_tensor("fxstat", [KX, P, P], BF16, kind="ExternalInput")
    d_fhs = nc.dram_tensor("fhstat", [KC, P, P], BF16, kind="ExternalInput")
    d_bxi = nc.dram_tensor("b_xi", [3, P], F32, kind="ExternalInput")
    d_biou = nc.dram_tensor("b_iou", [3, P], F32, kind="ExternalInput")
    d_bxf = nc.dram_tensor("b_xf", [P], F32, kind="ExternalInput")
    d_bfh = nc.dram_tensor("b_fh", [P], F32, kind="ExternalInput")
    d_hout = nc.dram_tensor("hout", [P, C], F32, kind="ExternalOutput")

    ACT = mybir.ActivationFunctionType
    NMAXC = max([i["ncols"] for i in wave_info] + [1])
    NCHMX = K * NMAXC

    with tile.TileContext(nc, num_cores=NCORES) as tc:
        with (
            tc.tile_pool(name="const", bufs=1) as cpool,
            tc.tile_pool(name="state", bufs=1) as spool,
            tc.tile_pool(name="wstage", bufs=6) as wpool,
            tc.tile_pool(name="work", bufs=1) as wk,
            tc.tile_pool(name="psum", bufs=1, space="PSUM") as pp,
            tc.tile_pool(name="psg", bufs=1, space="PSUM") as pg,
            tc.tile_pool(name="dram", bufs=2, space="DRAM") as dp,
        ):
            # ---- warmup collective (absorbs mesh setup under compute) -----
            wu = cpool.tile([P, 8], BF16)
            nc.vector.memset(wu[:], 0.0)
            wu_in = dp.tile([P, 8], BF16, tag="wu_in")
            nc.sync.dma_start(wu_in[:], wu[:])
            wu_out = dp.tile([NCORES, P, 8], BF16, tag="wu_out",
                             addr_space="Shared")
            nc.gpsimd.collective_compute(
                "AllGather", mybir.AluOpType.bypass,
                ins=[wu_in.opt()], outs=[wu_out.opt()],
                replica_groups=[list(range(NCORES))])

            # ---- constants -------------------------------------------------
            xt = cpool.tile([P, KX, C], BF16)
            nc.sync.dma_start(xt[:], d_xt.ap().rearrange("k p n -> p k n"))
            iouxs = cpool.tile([P, KX * 3, P], BF16)
            nc.sync.dma_start(iouxs[:], d_iouxs.ap().rearrange("s p m -> p s m"))
            iouhs = cpool.tile([P, KC * 3, P], BF16)
            nc.sync.dma_start(iouhs[:], d_iouhs.ap().rearrange("s p m -> p s m"))
            fxs = cpool.tile([P, KX, P], BF16)
            nc.sync.dma_start(fxs[:], d_fxs.ap().rearrange("s p m -> p s m"))
            fhs = cpool.tile([P, KC, P], BF16)
            nc.sync.dma_start(fhs[:], d_fhs.ap().rearrange("s p m -> p s m"))
            bxi = cpool.tile([P, 3], F32)
            nc.sync.dma_start(bxi[:], d_bxi.ap().rearrange("g p -> p g"))
            biou = cpool.tile([P, 3], F32)
            nc.sync.dma_start(biou[:], d_biou.ap().rearrange("g p -> p g"))
            bxf = cpool.tile([P, 1], F32)
            nc.sync.dma_start(bxf[:], d_bxf.ap().rearrange("(p one) -> p one", one=1))
            bfh = cpool.tile([P, 1], F32)
            nc.sync.dma_start(bfh[:], d_bfh.ap().rearrange("(p one) -> p one", one=1))
            mbuf = cpool.tile([P, max(MTK, 1)], BF16)
            nc.sync.dma_start(mbuf[:], d_mask.ap())

            # weight slot tiles; DMAs emitted upfront, prefetch pipelined
            wtile = []
            for s in range(S_total):
                t = wpool.tile([P, KC * KC, P], BF16, tag="wst")
                nc.sync.dma_start(t[:], d_ws.ap()[s])
                wtile.append(t)

            # ---- state -----------------------------------------------------
            h_bf = spool.tile([P, KC, NPAD], BF16)
            nc.vector.memset(h_bf[:], 0.0)
            c_sl = spool.tile([P, NPAD], F32)
            nc.vector.memset(c_sl[:], 0.0)
            h_sl = spool.tile([P, C], F32)
            xi_f = spool.tile([P, 3, C], F32)
            xf_f = spool.tile([P, C], F32)

            # ---- xi/xf precompute (column-sharded feature slice) -----------
            CCH = 128
            for cc in range(0, C, CCH):
                ncc = min(CCH, C - cc)
                ps = pg.tile([P, 3, CCH], F32, tag="ps3")
                for g in range(3):
                    for k in range(KX):
                        nc.tensor.matmul(
                            ps[:, g, :ncc],
                            iouxs[:, k * 3 + g, :],
                            xt[:, k, cc:cc + ncc],
                            start=(k == 0), stop=(k == KX - 1))
                for g in range(3):
                    nc.scalar.activation(
                        xi_f[:, g, cc:cc + ncc], ps[:, g, :ncc],
                        ACT.Identity, bias=bxi[:, g:g + 1])
                psf0 = pg.tile([P, CCH], F32, tag="psf0")
                for k in range(KX):
                    nc.tensor.matmul(
                        psf0[:, :ncc], fxs[:, k, :], xt[:, k, cc:cc + ncc],
                        start=(k == 0), stop=(k == KX - 1))
                nc.scalar.activation(
                    xf_f[:, cc:cc + ncc], psf0[:, :ncc],
                    ACT.Identity, bias=bxf[:, 0:1])

            def publish_h(p0, n, tag):
                hb = wk.tile([P, n], BF16, tag="hb" + tag)
                nc.vector.tensor_copy(hb[:, :n], h_sl[:, p0:p0 + n])
                gin = dp.tile([P, n], BF16, tag="gin" + tag)
                nc.sync.dma_start(gin[:], hb[:])
                gout = dp.tile([NCORES, P, n], BF16, tag="gout" + tag,
                               addr_space="Shared")
                nc.gpsimd.collective_compute(
                    "AllGather", mybir.AluOpType.bypass,
                    ins=[gin.opt()], outs=[gout.opt()],
                    replica_groups=[list(range(NCORES))])
                nc.sync.dma_start(
                    h_bf[:, :, p0:p0 + n],
                    gout[:, :, :n].rearrange("k p n -> p k n"))

            # ---- wave 0: leaves (iou = xi, no children) --------------------
            for cc in range(0, n0, NMAXC):
                L = min(NMAXC, n0 - cc)
                ig = wk.tile([P, NMAXC], F32, tag="ig")
                og = wk.tile([P, NMAXC], F32, tag="og")
                ug = wk.tile([P, NMAXC], F32, tag="ug")
                nc.scalar.activation(ig[:, :L], xi_f[:, 0, cc:cc + L],
                                     ACT.Sigmoid, bias=biou[:, 0:1])
                nc.scalar.activation(og[:, :L], xi_f[:, 1, cc:cc + L],
                                     ACT.Sigmoid, bias=biou[:, 1:2])
                nc.scalar.activation(ug[:, :L], xi_f[:, 2, cc:cc + L],
                                     ACT.Tanh, bias=biou[:, 2:3])
                nc.vector.tensor_mul(c_sl[:, cc:cc + L], ig[:, :L], ug[:, :L])
                tct = wk.tile([P, NMAXC], F32, tag="tct")
                nc.scalar.activation(tct[:, :L], c_sl[:, cc:cc + L], ACT.Tanh)
                nc.vector.tensor_mul(h_sl[:, cc:cc + L], og[:, :L], tct[:, :L])
            publish_h(0, n0, "0")

            # ---- internal waves --------------------------------------------
            for wi, info in enumerate(wave_info):
                kind = info["kind"]
                base, ncols = info["base"], info["ncols"]
                nch = K * ncols
                last = wi == len(wave_info) - 1

                # gather child cols: h (DVE) + c (gpsimd, needed at gates)
                hch = wk.tile([P, KC, NCHMX], BF16, tag="hch")
                ccg = wk.tile([P, NCHMX], F32, tag="ccg")
                if info["has_missing"]:
                    nc.vector.memset(hch[:, :, :nch], 0.0)
                    nc.gpsimd.memset(ccg[:, :nch], 0.0)
                for (dst, src, ln) in info["runs"]:
                    nc.vector.tensor_copy(hch[:, :, dst:dst + ln],
                                          h_bf[:, :, src:src + ln])
                for (dst, src, ln) in info["runs"]:
                    nc.gpsimd.tensor_copy(ccg[:, dst:dst + ln],
                                          c_sl[:, src:src + ln])

                hsum_f = wk.tile([P, KC, NMAXC], F32, tag="hsumf")
                nc.vector.tensor_reduce(
                    hsum_f[:, :, :ncols],
                    hch[:, :, :nch].rearrange("p k (n c) -> p k n c", c=K),
                    axis=mybir.AxisListType.X, op=mybir.AluOpType.add)
                hsum_b = wk.tile([P, KC, NMAXC], BF16, tag="hsumb")
                nc.vector.tensor_copy(hsum_b[:, :, :ncols], hsum_f[:, :, :ncols])

                if kind == "ident":
                    rhs = hsum_b
                elif kind == "repl":
                    psr = pp.tile([P, KC, P], F32, tag="psr")
                    wst = wtile[info["soff"]]
                    for m in range(KC):
                        for k in range(KC):
                            nc.tensor.matmul(
                                psr[:, m, :ncols],
                                wst[:, m * KC + k, :],
                                hsum_b[:, k, :ncols],
                                start=(k == 0), stop=(k == KC - 1))
                    chs = wk.tile([P, KC, NMAXC], BF16, tag="chs")
                    nc.scalar.activation(
                        chs[:, :, 0:1].rearrange("p k one -> p (k one)"),
                        psr[:, :, 0:1].rearrange("p k one -> p (k one)"),
                        ACT.Identity)
                    rhs = chs
                else:
                    ns, nmax = info["ns"], info["nmax"]
                    soff, moff = info["soff"], info["moff"]
                    psr = pp.tile([P, KC, P], F32, tag="psr")
                    msels = []
                    for s in range(ns):
                        msel = wk.tile([P, KC, NMAXC], BF16,
                                       tag="msel" + str(s))
                        mo = moff + s * KC * ncols
                        nc.vector.tensor_mul(
                            msel[:, :, :ncols], hsum_b[:, :, :ncols],
                            mbuf[:, mo:mo + KC * ncols].rearrange(
                                "p (k n) -> p k n", k=KC))
                        msels.append(msel)
                    for m in range(KC):
                        for s in range(ns):
                            wst = wtile[soff + s]
                            for k in range(KC):
                                nc.tensor.matmul(
                                    psr[:, m, :ncols],
                                    wst[:, m * KC + k, :],
                                    msels[s][:, k, :ncols],
                                    start=(s == 0 and k == 0),
                                    stop=(s == ns - 1 and k == KC - 1))
                    # pack own group's columns: sum over group axis (others 0)
                    tg = str(wi)
                    csp = wk.tile([P, KC * nmax], F32, tag="csp" + tg)
                    nc.vector.tensor_reduce(
                        csp[:].rearrange("p (k t) -> p k t", k=KC),
                        psr[:, :, :ncols].rearrange("p k (g t) -> p k t g",
                                                    t=nmax),
                        axis=mybir.AxisListType.X, op=mybir.AluOpType.add)
                    csb = wk.tile([P, KC * nmax], BF16, tag="csb" + tg)
                    nc.scalar.activation(csb[:], csp[:], ACT.Identity)
                    gin = dp.tile([P, KC, nmax], BF16, tag="gcs_in" + tg)
                    nc.sync.dma_start(
                        gin[:], csb[:].rearrange("p (k t) -> p k t", k=KC))
                    gout = dp.tile([NCORES, P, KC, nmax], BF16,
                                   tag="gcs_out" + tg, addr_space="Shared")
                    nc.gpsimd.collective_compute(
                        "AllGather", mybir.AluOpType.bypass,
                        ins=[gin.opt()], outs=[gout.opt()],
                        replica_groups=[list(range(NCORES))])
                    chs = wk.tile([P, KC, NMAXC], BF16, tag="chs")
                    for g in range(NCORES):
                        nc.sync.dma_start(
                            chs[:, :, g * nmax:(g + 1) * nmax], gout[g])
                    rhs = chs

                # fh matmuls (independent of ch_sum -> overlap the AG)
                psf = pg.tile([P, NCHMX], F32, tag="psf")
                for k in range(KC):
                    nc.tensor.matmul(
                        psf[:, :nch], fhs[:, k, :], hch[:, k, :nch],
                        start=(k == 0), stop=(k == KC - 1))

                # iou matmuls (column-sharded)
                psi = pg.tile([P, 3, P], F32, tag="psi")
                for g in range(3):
                    for k in range(KC):
                        nc.tensor.matmul(
                            psi[:, g, :ncols], iouhs[:, k * 3 + g, :],
                            rhs[:, k, :ncols],
                            start=(k == 0), stop=(k == KC - 1))

                # gates
                n = ncols
                tmp = wk.tile([P, 3, NMAXC], F32, tag="gtmp")
                nc.vector.tensor_add(tmp[:, :, :n], psi[:, :, :n],
                                     xi_f[:, :, base:base + n])
                ig = wk.tile([P, NMAXC], F32, tag="ig")
                og = wk.tile([P, NMAXC], F32, tag="og")
                ug = wk.tile([P, NMAXC], F32, tag="ug")
                nc.scalar.activation(ig[:, :n], tmp[:, 0, :n], ACT.Sigmoid,
                                     bias=biou[:, 0:1])
                nc.scalar.activation(og[:, :n], tmp[:, 1, :n], ACT.Sigmoid,
                                     bias=biou[:, 1:2])
                nc.scalar.activation(ug[:, :n], tmp[:, 2, :n], ACT.Tanh,
                                     bias=biou[:, 2:3])
                cn = wk.tile([P, NMAXC], F32, tag="cn")
                nc.vector.tensor_mul(cn[:, :n], ig[:, :n], ug[:, :n])
                # f = sigmoid(fh + xf[parent] + b); fc = sum_k f*cc
                fsb = wk.tile([P, NCHMX], F32, tag="fsb")
                xfb = wk.tile([P, NCHMX], F32, tag="xfb")
                xfb_v = xfb[:, :nch].rearrange("p (n k) -> p n k", k=K)
                for kk in range(K):
                    nc.vector.tensor_copy(
                        xfb_v[:, :, kk:kk + 1],
                        xf_f[:, base:base + n].rearrange(
                            "p (n one) -> p n one", one=1))
                nc.vector.tensor_add(fsb[:, :nch], psf[:, :nch], xfb[:, :nch])
                nc.scalar.activation(fsb[:, :nch], fsb[:, :nch],
                                     ACT.Sigmoid, bias=bfh[:, 0:1])
                nc.vector.tensor_mul(fsb[:, :nch], fsb[:, :nch], ccg[:, :nch])
                fc = wk.tile([P, NMAXC], F32, tag="fc")
                nc.vector.tensor_reduce(
                    fc[:, :n],
                    fsb[:, :nch].rearrange("p (n k) -> p n k", k=K),
                    axis=mybir.AxisListType.X, op=mybir.AluOpType.add)
                nc.vector.tensor_add(cn[:, :n], cn[:, :n], fc[:, :n])
                nc.vector.tensor_copy(c_sl[:, base:base + n], cn[:, :n])
                tct = wk.tile([P, NMAXC], F32, tag="tct")
                nc.scalar.activation(tct[:, :n], cn[:, :n], ACT.Tanh)
                nc.vector.tensor_mul(h_sl[:, base:base + n], og[:, :n],
                                     tct[:, :n])
                if not last:
                    publish_h(base, ncols, str(wi + 1))

            nc.sync.dma_start(d_hout.ap(), h_sl[:])

    in_maps = []
    for c in range(NCORES):
        in_maps.append({
            "wstream": wstream[c], "masks": maskbuf[c],
            "xt": xT_b, "iouxstat": iouxstat[c], "iouhstat": iouhstat[c],
            "fxstat": fxstat[c], "fhstat": fhstat[c],
            "b_xi": b_xi[c], "b_iou": b_iou[c], "b_xf": b_xf[c],
            "b_fh": b_fh[c],
        })
    _split_multi_waits(nc)
    kernel._nc = nc
    kernel._in_maps = in_maps
    res = run_bass_kernel_spmd(nc, in_maps, list(range(NCORES)))
    hT = np.concatenate([res.results[c]["hout"] for c in range(NCORES)], 0)
    out = np.empty((N, MEM), np.float32)
    for node in range(N):
        out[node] = hT[:, col_of[node]]
    return out


# revision 22
# speedup vs baseline: 1.0170x; 1.0170x over previous
"""ChildSumTreeLSTM with relation transforms on 8 Trainium2 NeuronCores.

Layout: transposed (features on SBUF partitions, tree nodes on the free dim),
feature-sharded state (each core owns a 128-feature slice of h/c/xi/gates).
Column order: wave 0 (leaves) in heap order; each internal wave's parent
columns are grouped by the core that owns their relation (8 equal padded
groups) so every cross-core exchange is an AllGather of the core's own
contiguous shard — no AllReduce anywhere:

  per wave: gather child cols -> hsum -> per-slot (rel) masked-rhs matmuls
  accumulated into one PSUM region (mask selects that rel's columns; the
  per-core psum is then nonzero only on the core's own column group) ->
  group-axis reduce packs the AG shard -> AllGather ch_sum -> column-sharded
  iou gates + f gates -> AllGather of the new h feature-slices.

The wave-4 single node's relation matrix is replicated on all cores (skips
its ch_sum exchange); the root uses the identity relation (no matmul).
A dummy warmup collective at program start absorbs the first-collective
mesh setup latency under the xi/xf matmuls.
All per-core differences are input data (weight slots, masks, bias slices),
so one Bass program runs SPMD on all 8 cores.
"""

import sys

sys.path.insert(0, "/opt/trn_rl_repo")

import numpy as np
import ml_dtypes

import concourse.bass as bass
import concourse.mybir as mybir
import concourse.tile as tile
from concourse.bass_utils import run_bass_kernel_spmd
from concourse.vector_clock import ScopedClock, VectorClock

BF16 = mybir.dt.bfloat16
FP8 = mybir.dt.float8e4
F32 = mybir.dt.float32
NCORES = 8
P = 128
WSCALE = 64.0  # Wrel stored as fp8 * WSCALE; 1/WSCALE folded into iouh_w


# This walrus build rejects >1 sem wait per instruction at the Tile exit
# drain; split the aggregated drain into one drain per proc.
def _split_drain_and_barrier(self, tick_clock, wait_clock):
    gc = tick_clock.global_clock
    n = len(gc)
    nonzero = [i for i in range(n) if gc[i] > 0]
    for j in nonzero:
        vec = VectorClock([gc[i] if i == j else 0 for i in range(n)])
        d = self.nc.sync.drain()
        wait_clock.add_sem_waits(d.ins, ScopedClock({None: vec}))
    if not nonzero:
        d = self.nc.sync.drain()
        wait_clock.add_sem_waits(d.ins, ScopedClock({None: gc.copy()}))
    self.nc.all_engine_barrier()
    assert self.sems is not None
    popped = self.nc._tile_sem_poison_stack.pop()
    assert popped is self._sem_poison
    self.nc.clear_and_free_semaphores(list(self.sems.allocated().values()))
    self.nc.all_engine_barrier()


tile.TileContext._drain_and_barrier = _split_drain_and_barrier


def _split_multi_waits(nc, limit=1):
    """Walrus here allows only one sem wait per instruction; hoist extras
    onto same-engine NOPs inserted right before the instruction."""
    for bb in nc.main_func.blocks:
        new_list = []
        for ins in bb.instructions:
            si = getattr(ins, "sync_info", None)
            if si is not None and si.on_wait and len(si.on_wait) > limit:
                waits = list(si.on_wait)
                for w in waits[:-limit]:
                    nop = mybir.InstNoOp(
                        name=nc.get_next_instruction_name(),
                        sync_info=mybir.SyncInfo(on_wait=[w], on_update=[]),
                        bass_nofuse=True,
                        engine=ins.engine,
                    )
                    nc.register_instruction(nop, overwrite=True)
                    new_list.append(nop)
                si.on_wait = waits[-limit:]
            new_list.append(ins)
        bb.instructions[:] = new_list


def _bf16(a):
    return np.ascontiguousarray(a.astype(ml_dtypes.bfloat16))


def _blocksT(mat):
    """[M, K] -> [K/128 * M/128, 128, 128] of transposed blocks, grouped as
    [m, k] -> index m*KC + k, each block = mat[mb, kb].T (lhsT)."""
    M, K = mat.shape
    MC, KC = M // P, K // P
    out = np.empty((MC * KC, P, P), mat.dtype)
    for m in range(MC):
        for k in range(KC):
            out[m * KC + k] = mat[m * P:(m + 1) * P, k * P:(k + 1) * P].T
    return out


def _runs(seq, zcol):
    """Maximal +1-contiguous runs of seq, skipping zcol entries.
    Returns list of (dst_off, src_col, length)."""
    runs = []
    i0 = 0
    n = len(seq)
    while i0 < n:
        if seq[i0] == zcol:
            i0 += 1
            continue
        i1 = i0 + 1
        while i1 < n and seq[i1] == seq[i1 - 1] + 1 and seq[i1] != zcol:
            i1 += 1
        runs.append((i0, int(seq[i0]), i1 - i0))
        i0 = i1
    return runs


def _plan(child_idx, rel_ids, Wrel):
    """Host-side planning: waves, rel->core assignment, grouped column
    order, per-wave child-gather runs."""
    N, K = child_idx.shape
    R1 = Wrel.shape[0]
    eff_children = []
    wave = np.zeros(N, np.int32)
    for i in range(N):
        cs = [int(c) for c in child_idx[i] if 0 <= c < i]
        eff_children.append(cs)
        wave[i] = 1 + max((wave[c] for c in cs), default=-1)
    nwaves = int(wave.max()) + 1

    ident = set()
    eye = np.eye(Wrel.shape[1], dtype=Wrel.dtype)
    for r in set(int(rel_ids[i]) for i in range(N)):
        if np.array_equal(Wrel[r], eye):
            ident.add(r)

    wave_nodes = [sorted([i for i in range(N) if wave[i] == w], key=lambda i: -i)
                  for w in range(nwaves)]

    # wave kinds: 0 = leaves; 'shard' = sharded rel + AGcs; 'repl' =
    # replicated weights (tiny waves); 'ident' = identity rel only
    kinds = []
    for w in range(1, nwaves):
        nodes = wave_nodes[w]
        rels = set(int(rel_ids[i]) for i in nodes)
        if rels <= ident:
            kinds.append("ident")
        elif len(nodes) == 1:
            kinds.append("repl")
        else:
            kinds.append("shard")

    # per sharded wave: assign rels -> cores (balance #rels, then #cols)
    wave_info = []
    for w in range(1, nwaves):
        nodes = wave_nodes[w]
        kind = kinds[w - 1]
        info = dict(kind=kind, nodes=nodes)
        if kind == "shard":
            from collections import Counter
            cnt = Counter(int(rel_ids[i]) for i in nodes)
            rels = sorted(cnt, key=lambda r: -cnt[r])
            ns = (len(rels) + NCORES - 1) // NCORES
            core_rels = [[] for _ in range(NCORES)]
            core_cols = [0] * NCORES
            for r in rels:
                best = min(range(NCORES),
                           key=lambda c: (len(core_rels[c]) >= ns,
                                          core_cols[c], len(core_rels[c])))
                core_rels[best].append(r)
                core_cols[best] += cnt[r]
            nmax = max(core_cols)
            # grouped node order: per core, by (rel, -node); pad to nmax
            grouped = []  # per col: node or None
            for c in range(NCORES):
                cn = [i for i in nodes if int(rel_ids[i]) in core_rels[c]]
                cn.sort(key=lambda i: (int(rel_ids[i]), -i))
                grouped.extend(cn)
                grouped.extend([None] * (nmax - len(cn)))
            info.update(ns=ns, core_rels=core_rels, nmax=nmax,
                        grouped=grouped, ncols=NCORES * nmax)
        else:
            info.update(grouped=list(nodes), ncols=len(nodes))
        wave_info.append(info)

    # global column order
    col_of = np.full(N, -1, np.int64)
    order_cols = []  # per col: node or None
    for i in wave_nodes[0]:
        col_of[i] = len(order_cols)
        order_cols.append(i)
    bases = [0]
    for info in wave_info:
        info["base"] = len(order_cols)
        bases.append(info["base"])
        for node in info["grouped"]:
            if node is not None:
                col_of[node] = len(order_cols)
            order_cols.append(node)
    C = len(order_cols)
    ZCOL = C
    NPAD = C + 6

    # child gather runs per wave (over grouped parent order; h and c share)
    for info in wave_info:
        seq = []
        for node in info["grouped"]:
            if node is None:
                seq.extend([ZCOL] * K)
            else:
                cs = eff_children[node]
                seq.extend([int(col_of[c]) for c in cs])
                seq.extend([ZCOL] * (K - len(cs)))
        info["runs"] = _runs(seq, ZCOL)
        info["has_missing"] = any(s == ZCOL for s in seq)

    return dict(wave=wave, nwaves=nwaves, wave_nodes=wave_nodes,
                wave_info=wave_info, col_of=col_of, order_cols=order_cols,
                C=C, ZCOL=ZCOL, NPAD=NPAD, ident=ident,
                eff_children=eff_children)


def kernel(**inputs):
    x = np.asarray(inputs["x"], np.float32)
    Wrel = np.asarray(inputs["Wrel"], np.float32)
    ioux_w = np.asarray(inputs["ioux_w"], np.float32)
    ioux_b = np.asarray(inputs["ioux_b"], np.float32)
    iouh_w = np.asarray(inputs["iouh_w"], np.float32)
    iouh_b = np.asarray(inputs["iouh_b"], np.float32)
    fx_w = np.asarray(inputs["fx_w"], np.float32)
    fx_b = np.asarray(inputs["fx_b"], np.float32)
    fh_w = np.asarray(inputs["fh_w"], np.float32)
    fh_b = np.asarray(inputs["fh_b"], np.float32)
    child_idx = np.asarray(inputs["child_idx"], np.int32)
    rel_ids = np.asarray(inputs["rel_ids"], np.int32)

    N, IN_DIM = x.shape
    MEM = fh_w.shape[0]
    KC = MEM // P            # 8 feature chunks
    KX = IN_DIM // P         # 8 input chunks
    K = child_idx.shape[1]   # max children (4)

    plan = _plan(child_idx, rel_ids, Wrel)
    wave_info, col_of = plan["wave_info"], plan["col_of"]
    order_cols, C, ZCOL, NPAD = plan["order_cols"], plan["C"], plan["ZCOL"], plan["NPAD"]
    n0 = len(plan["wave_nodes"][0])

    # ---- per-core host data -------------------------------------------------
    xT = np.zeros((IN_DIM, C), np.float32)
    for j, node in enumerate(order_cols):
        if node is not None:
            xT[:, j] = x[node]
    xT_b = np.zeros((KX, P, C), ml_dtypes.bfloat16)
    for k in range(KX):
        xT_b[k] = _bf16(xT[k * P:(k + 1) * P])

    # weight slots + masks
    shard_waves = [i for i in wave_info if i["kind"] == "shard"]
    repl_waves = [i for i in wave_info if i["kind"] == "repl"]
    S_total = sum(i["ns"] for i in shard_waves) + len(repl_waves)
    MTK = sum(i["ns"] * KC * i["ncols"] for i in shard_waves)

    wstream = [np.zeros((S_total, P, KC * KC, P), ml_dtypes.float8_e4m3)
               for _ in range(NCORES)]
    maskbuf = [np.zeros((P, max(MTK, 1)), ml_dtypes.bfloat16)
               for _ in range(NCORES)]
    soff = 0
    moff = 0
    for info in wave_info:
        if info["kind"] == "shard":
            base, nmax, ncols = info["base"], info["nmax"], info["ncols"]
            info["soff"], info["moff"] = soff, moff
            for c in range(NCORES):
                for s, r in enumerate(info["core_rels"][c]):
                    wstream[c][soff + s] = (
                        _blocksT(Wrel[r]).transpose(1, 0, 2) * WSCALE
                    ).astype(ml_dtypes.float8_e4m3)
                    m = np.zeros((KC, ncols), np.float32)
                    for t in range(ncols):
                        node = info["grouped"][t]
                        if node is not None and int(rel_ids[node]) == r:
                            m[:, t] = 1.0
                    mo = moff + s * KC * ncols
                    maskbuf[c][:, mo:mo + KC * ncols] = _bf16(
                        np.broadcast_to(m.reshape(1, -1), (P, KC * ncols)))
            soff += info["ns"]
            moff += info["ns"] * KC * ncols
        elif info["kind"] == "repl":
            info["soff"] = soff
            r = int(rel_ids[info["nodes"][0]])
            wb = (_blocksT(Wrel[r]).transpose(1, 0, 2) * WSCALE).astype(
                ml_dtypes.float8_e4m3)
            for c in range(NCORES):
                wstream[c][soff] = wb
            soff += 1

    iouxstat = [np.zeros((KX * 3, P, P), ml_dtypes.bfloat16) for _ in range(NCORES)]
    iouhstat = [np.zeros((KC * 3, P, P), ml_dtypes.bfloat16) for _ in range(NCORES)]
    fxstat = [np.zeros((KX, P, P), ml_dtypes.bfloat16) for _ in range(NCORES)]
    fhstat = [np.zeros((KC, P, P), ml_dtypes.bfloat16) for _ in range(NCORES)]
    b_xi = [np.zeros((3, P), np.float32) for _ in range(NCORES)]
    b_iou = [np.zeros((3, P), np.float32) for _ in range(NCORES)]
    b_xf = [np.zeros((P,), np.float32) for _ in range(NCORES)]
    b_fh = [np.zeros((P,), np.float32) for _ in range(NCORES)]
    for c in range(NCORES):
        rows = slice(c * P, (c + 1) * P)
        for g in range(3):
            gr = slice(g * MEM + c * P, g * MEM + (c + 1) * P)
            b_xi[c][g] = ioux_b[gr]
            b_iou[c][g] = iouh_b[gr]
            for k in range(KX):
                iouxstat[c][k * 3 + g] = _bf16(ioux_w[gr, k * P:(k + 1) * P].T)
            for k in range(KC):
                iouhstat[c][k * 3 + g] = _bf16(
                    iouh_w[gr, k * P:(k + 1) * P].T / WSCALE)
        b_xf[c] = fx_b[rows]
        b_fh[c] = fh_b[rows]
        for k in range(KX):
            fxstat[c][k] = _bf16(fx_w[rows, k * P:(k + 1) * P].T)
        for k in range(KC):
            fhstat[c][k] = _bf16(fh_w[rows, k * P:(k + 1) * P].T)

    # ---- build program ------------------------------------------------------
    nc = bass.Bass("TRN2", target_bir_lowering=False, debug=False,
                   num_devices=NCORES)
    d_ws = nc.dram_tensor("wstream", list(wstream[0].shape), FP8,
                          kind="ExternalInput")
    d_mask = nc.dram_tensor("masks", list(maskbuf[0].shape), BF16,
                            kind="ExternalInput")
    d_xt = nc.dram_tensor("xt", [KX, P, C], BF16, kind="ExternalInput")
    d_iouxs = nc.dram_tensor("iouxstat", [KX * 3, P, P], BF16, kind="ExternalInput")
    d_iouhs = nc.dram_tensor("iouhstat", [KC * 3, P, P], BF16, kind="ExternalInput")
    d_fxs = nc.dram_tensor("fxstat", [KX, P, P], BF16, kind="ExternalInput")
    d_fhs = nc.dram_tensor("fhstat", [KC, P, P], BF16, kind="ExternalInput")
    d_bxi = nc.dram_tensor("b_xi", [3, P], F32, kind="ExternalInput")
    d_biou = nc.dram_tensor("b_iou", [3, P], F32, kind="ExternalInput")
    d_bxf = nc.dram_tensor("b_xf", [P], F32, kind="ExternalInput")
    d_bfh = nc.dram_tensor("b_fh", [P], F32, kind="ExternalInput")
    d_hout = nc.dram_tensor("hout", [P, C], F32, kind="ExternalOutput")

    ACT = mybir.ActivationFunctionType
    NMAXC = max([i["ncols"] for i in wave_info] + [1])
    NCHMX = K * NMAXC

    with tile.TileContext(nc, num_cores=NCORES) as tc:
        with (
            tc.tile_pool(name="const", bufs=1) as cpool,
            tc.tile_pool(name="state", bufs=1) as spool,
            tc.tile_pool(name="wstage", bufs=10) as wpool,
            tc.tile_pool(name="work", bufs=1) as wk,
            tc.tile_pool(name="psum", bufs=1, space="PSUM") as pp,
            tc.tile_pool(name="psg", bufs=1, space="PSUM") as pg,
            tc.tile_pool(name="dram", bufs=2, space="DRAM") as dp,
        ):
            # ---- warmup collective (absorbs mesh setup under compute) -----
            wu = cpool.tile([P, 8], BF16)
            nc.vector.memset(wu[:], 0.0)
            wu_in = dp.tile([P, 8], BF16, tag="wu_in")
            nc.sync.dma_start(wu_in[:], wu[:])
            wu_out = dp.tile([NCORES, P, 8], BF16, tag="wu_out",
                             addr_space="Shared")
            nc.gpsimd.collective_compute(
                "AllGather", mybir.AluOpType.bypass,
                ins=[wu_in.opt()], outs=[wu_out.opt()],
                replica_groups=[list(range(NCORES))])

            # ---- constants -------------------------------------------------
            xt = cpool.tile([P, KX, C], BF16)
            nc.sync.dma_start(xt[:], d_xt.ap().rearrange("k p n -> p k n"))
            iouxs = cpool.tile([P, KX * 3, P], BF16)
            nc.sync.dma_start(iouxs[:], d_iouxs.ap().rearrange("s p m -> p s m"))
            iouhs = cpool.tile([P, KC * 3, P], BF16)
            nc.sync.dma_start(iouhs[:], d_iouhs.ap().rearrange("s p m -> p s m"))
            fxs = cpool.tile([P, KX, P], BF16)
            nc.sync.dma_start(fxs[:], d_fxs.ap().rearrange("s p m -> p s m"))
            fhs = cpool.tile([P, KC, P], BF16)
            nc.sync.dma_start(fhs[:], d_fhs.ap().rearrange("s p m -> p s m"))
            bxi = cpool.tile([P, 3], F32)
            nc.sync.dma_start(bxi[:], d_bxi.ap().rearrange("g p -> p g"))
            biou = cpool.tile([P, 3], F32)
            nc.sync.dma_start(biou[:], d_biou.ap().rearrange("g p -> p g"))
            bxf = cpool.tile([P, 1], F32)
            nc.sync.dma_start(bxf[:], d_bxf.ap().rearrange("(p one) -> p one", one=1))
            bfh = cpool.tile([P, 1], F32)
            nc.sync.dma_start(bfh[:], d_bfh.ap().rearrange("(p one) -> p one", one=1))
            mbuf = cpool.tile([P, max(MTK, 1)], BF16)
            nc.sync.dma_start(mbuf[:], d_mask.ap())

            # weight slot tiles on the ACT DMA queue so the bulk stream
            # never blocks the SP queue feeding the collectives
            wtile = []
            for s in range(S_total):
                t = wpool.tile([P, KC * KC, P], FP8, tag="wst")
                nc.scalar.dma_start(t[:], d_ws.ap()[s])
                wtile.append(t)

            # ---- state -----------------------------------------------------
            h_bf = spool.tile([P, KC, NPAD], BF16)
            nc.vector.memset(h_bf[:], 0.0)
            c_sl = spool.tile([P, NPAD], F32)
            nc.vector.memset(c_sl[:], 0.0)
            h_sl = spool.tile([P, C], F32)
            xi_f = spool.tile([P, 3, C], F32)
            xf_f = spool.tile([P, C], F32)

            # ---- xi/xf precompute (column-sharded feature slice) -----------
            CCH = 128
            for cc in range(0, C, CCH):
                ncc = min(CCH, C - cc)
                ps = pg.tile([P, 3, CCH], F32, tag="ps3")
                for g in range(3):
                    for k in range(KX):
                        nc.tensor.matmul(
                            ps[:, g, :ncc],
                            iouxs[:, k * 3 + g, :],
                            xt[:, k, cc:cc + ncc],
                            start=(k == 0), stop=(k == KX - 1))
                for g in range(3):
                    nc.scalar.activation(
                        xi_f[:, g, cc:cc + ncc], ps[:, g, :ncc],
                        ACT.Identity, bias=bxi[:, g:g + 1])
                psf0 = pg.tile([P, CCH], F32, tag="psf0")
                for k in range(KX):
                    nc.tensor.matmul(
                        psf0[:, :ncc], fxs[:, k, :], xt[:, k, cc:cc + ncc],
                        start=(k == 0), stop=(k == KX - 1))
                nc.scalar.activation(
                    xf_f[:, cc:cc + ncc], psf0[:, :ncc],
                    ACT.Identity, bias=bxf[:, 0:1])

            def publish_h(p0, n, tag):
                hb = wk.tile([P, n], BF16, tag="hb" + tag)
                nc.vector.tensor_copy(hb[:, :n], h_sl[:, p0:p0 + n])
                gin = dp.tile([P, n], BF16, tag="gin" + tag)
                nc.sync.dma_start(gin[:], hb[:])
                gout = dp.tile([NCORES, P, n], BF16, tag="gout" + tag,
                               addr_space="Shared")
                nc.gpsimd.collective_compute(
                    "AllGather", mybir.AluOpType.bypass,
                    ins=[gin.opt()], outs=[gout.opt()],
                    replica_groups=[list(range(NCORES))])
                nc.sync.dma_start(
                    h_bf[:, :, p0:p0 + n],
                    gout[:, :, :n].rearrange("k p n -> p k n"))

            # ---- wave 0: leaves (iou = xi, no children) --------------------
            for cc in range(0, n0, NMAXC):
                L = min(NMAXC, n0 - cc)
                ig = wk.tile([P, NMAXC], F32, tag="ig")
                og = wk.tile([P, NMAXC], F32, tag="og")
                ug = wk.tile([P, NMAXC], F32, tag="ug")
                nc.scalar.activation(ig[:, :L], xi_f[:, 0, cc:cc + L],
                                     ACT.Sigmoid, bias=biou[:, 0:1])
                nc.scalar.activation(og[:, :L], xi_f[:, 1, cc:cc + L],
                                     ACT.Sigmoid, bias=biou[:, 1:2])
                nc.scalar.activation(ug[:, :L], xi_f[:, 2, cc:cc + L],
                                     ACT.Tanh, bias=biou[:, 2:3])
                nc.vector.tensor_mul(c_sl[:, cc:cc + L], ig[:, :L], ug[:, :L])
                tct = wk.tile([P, NMAXC], F32, tag="tct")
                nc.scalar.activation(tct[:, :L], c_sl[:, cc:cc + L], ACT.Tanh)
                nc.vector.tensor_mul(h_sl[:, cc:cc + L], og[:, :L], tct[:, :L])
            publish_h(0, n0, "0")

            # ---- internal waves --------------------------------------------
            for wi, info in enumerate(wave_info):
                kind = info["kind"]
                base, ncols = info["base"], info["ncols"]
                nch = K * ncols
                last = wi == len(wave_info) - 1

                # gather child cols: h (DVE) + c (gpsimd, needed at gates)
                hch = wk.tile([P, KC, NCHMX], BF16, tag="hch")
                ccg = wk.tile([P, NCHMX], F32, tag="ccg")
                if info["has_missing"]:
                    nc.vector.memset(hch[:, :, :nch], 0.0)
                    nc.gpsimd.memset(ccg[:, :nch], 0.0)
                for ri, (dst, src, ln) in enumerate(info["runs"]):
                    if ri % 2 == 0:
                        nc.vector.tensor_copy(hch[:, :, dst:dst + ln],
                                              h_bf[:, :, src:src + ln])
                    else:
                        nc.scalar.copy(hch[:, :, dst:dst + ln],
                                       h_bf[:, :, src:src + ln])
                for (dst, src, ln) in info["runs"]:
                    nc.gpsimd.tensor_copy(ccg[:, dst:dst + ln],
                                          c_sl[:, src:src + ln])

                hsum_f = wk.tile([P, KC, NMAXC], F32, tag="hsumf")
                nc.vector.tensor_reduce(
                    hsum_f[:, :, :ncols],
                    hch[:, :, :nch].rearrange("p k (n c) -> p k n c", c=K),
                    axis=mybir.AxisListType.X, op=mybir.AluOpType.add)
                hsum_b = wk.tile([P, KC, NMAXC], BF16, tag="hsumb")
                nc.vector.tensor_copy(hsum_b[:, :, :ncols], hsum_f[:, :, :ncols])

                if kind == "ident":
                    # iouh carries 1/WSCALE; identity ch_sum must be prescaled
                    hs64 = wk.tile([P, KC, NMAXC], BF16, tag="hs64")
                    for k in range(KC):
                        nc.scalar.activation(hs64[:, k, :ncols],
                                             hsum_f[:, k, :ncols],
                                             ACT.Identity, scale=WSCALE)
                    rhs = hs64
                elif kind == "repl":
                    psr = pp.tile([P, KC, P], F32, tag="psr")
                    wst = wtile[info["soff"]]
                    for m in range(KC):
                        for k in range(KC):
                            nc.tensor.matmul(
                                psr[:, m, :ncols],
                                wst[:, m * KC + k, :],
                                hsum_b[:, k, :ncols],
                                start=(k == 0), stop=(k == KC - 1))
                    chs = wk.tile([P, KC, NMAXC], BF16, tag="chs")
                    nc.scalar.activation(
                        chs[:, :, 0:1].rearrange("p k one -> p (k one)"),
                        psr[:, :, 0:1].rearrange("p k one -> p (k one)"),
                        ACT.Identity)
                    rhs = chs
                else:
                    ns, nmax = info["ns"], info["nmax"]
                    soff, moff = info["soff"], info["moff"]
                    psr = pp.tile([P, KC, P], F32, tag="psr")
                    msels = []
                    for s in range(ns):
                        msel = wk.tile([P, KC, NMAXC], BF16,
                                       tag="msel" + str(s))
                        mo = moff + s * KC * ncols
                        nc.vector.tensor_mul(
                            msel[:, :, :ncols], hsum_b[:, :, :ncols],
                            mbuf[:, mo:mo + KC * ncols].rearrange(
                                "p (k n) -> p k n", k=KC))
                        msels.append(msel)
                    for m in range(KC):
                        for s in range(ns):
                            wst = wtile[soff + s]
                            for k in range(KC):
                                nc.tensor.matmul(
                                    psr[:, m, :ncols],
                                    wst[:, m * KC + k, :],
                                    msels[s][:, k, :ncols],
                                    start=(s == 0 and k == 0),
                                    stop=(s == ns - 1 and k == KC - 1))
                    # pack own group's columns: sum over group axis (others 0)
                    tg = str(wi)
                    csp = wk.tile([P, KC * nmax], F32, tag="csp" + tg)
                    nc.vector.tensor_reduce(
                        csp[:].rearrange("p (k t) -> p k t", k=KC),
                        psr[:, :, :ncols].rearrange("p k (g t) -> p k t g",
                                                    t=nmax),
                        axis=mybir.AxisListType.X, op=mybir.AluOpType.add)
                    csb = wk.tile([P, KC * nmax], BF16, tag="csb" + tg)
                    nc.scalar.activation(csb[:], csp[:], ACT.Identity)
                    gin = dp.tile([P, KC, nmax], BF16, tag="gcs_in" + tg)
                    nc.sync.dma_start(
                        gin[:], csb[:].rearrange("p (k t) -> p k t", k=KC))
                    gout = dp.tile([NCORES, P, KC, nmax], BF16,
                                   tag="gcs_out" + tg, addr_space="Shared")
                    nc.gpsimd.collective_compute(
                        "AllGather", mybir.AluOpType.bypass,
                        ins=[gin.opt()], outs=[gout.opt()],
                        replica_groups=[list(range(NCORES))])
                    chs = wk.tile([P, KC, NMAXC], BF16, tag="chs")
                    for g in range(NCORES):
                        eng = nc.sync if g % 2 == 0 else nc.scalar
                        eng.dma_start(
                            chs[:, :, g * nmax:(g + 1) * nmax], gout[g])
                    rhs = chs

                # fh matmuls (independent of ch_sum -> overlap the AG)
                psf = pg.tile([P, NCHMX], F32, tag="psf")
                for k in range(KC):
                    nc.tensor.matmul(
                        psf[:, :nch], fhs[:, k, :], hch[:, k, :nch],
                        start=(k == 0), stop=(k == KC - 1))

                # iou matmuls (column-sharded)
                psi = pg.tile([P, 3, P], F32, tag="psi")
                for g in range(3):
                    for k in range(KC):
                        nc.tensor.matmul(
                            psi[:, g, :ncols], iouhs[:, k * 3 + g, :],
                            rhs[:, k, :ncols],
                            start=(k == 0), stop=(k == KC - 1))

                # gates
                n = ncols
                tmp = wk.tile([P, 3, NMAXC], F32, tag="gtmp")
                nc.vector.tensor_add(tmp[:, :, :n], psi[:, :, :n],
                                     xi_f[:, :, base:base + n])
                ig = wk.tile([P, NMAXC], F32, tag="ig")
                og = wk.tile([P, NMAXC], F32, tag="og")
                ug = wk.tile([P, NMAXC], F32, tag="ug")
                nc.scalar.activation(ig[:, :n], tmp[:, 0, :n], ACT.Sigmoid,
                                     bias=biou[:, 0:1])
                nc.scalar.activation(og[:, :n], tmp[:, 1, :n], ACT.Sigmoid,
                                     bias=biou[:, 1:2])
                nc.scalar.activation(ug[:, :n], tmp[:, 2, :n], ACT.Tanh,
                                     bias=biou[:, 2:3])
                cn = wk.tile([P, NMAXC], F32, tag="cn")
                nc.vector.tensor_mul(cn[:, :n], ig[:, :n], ug[:, :n])
                # f = sigmoid(fh + xf[parent] + b); fc = sum_k f*cc
                fsb = wk.tile([P, NCHMX], F32, tag="fsb")
                xfb = wk.tile([P, NCHMX], F32, tag="xfb")
                xfb_v = xfb[:, :nch].rearrange("p (n k) -> p n k", k=K)
                for kk in range(K):
                    nc.vector.tensor_copy(
                        xfb_v[:, :, kk:kk + 1],
                        xf_f[:, base:base + n].rearrange(
                            "p (n one) -> p n one", one=1))
                nc.vector.tensor_add(fsb[:, :nch], psf[:, :nch], xfb[:, :nch])
                nc.scalar.activation(fsb[:, :nch], fsb[:, :nch],
                                     ACT.Sigmoid, bias=bfh[:, 0:1])
                nc.vector.tensor_mul(fsb[:, :nch], fsb[:, :nch], ccg[:, :nch])
                fc = wk.tile([P, NMAXC], F32, tag="fc")
                nc.vector.tensor_reduce(
                    fc[:, :n],
                    fsb[:, :nch].rearrange("p (n k) -> p n k", k=K),
                    axis=mybir.AxisListType.X, op=mybir.AluOpType.add)
                nc.vector.tensor_add(cn[:, :n], cn[:, :n], fc[:, :n])
                nc.vector.tensor_copy(c_sl[:, base:base + n], cn[:, :n])
                tct = wk.tile([P, NMAXC], F32, tag="tct")
                nc.scalar.activation(tct[:, :n], cn[:, :n], ACT.Tanh)
                nc.vector.tensor_mul(h_sl[:, base:base + n], og[:, :n],
                                     tct[:, :n])
                if not last:
                    publish_h(base, ncols, str(wi + 1))

            nc.sync.dma_start(d_hout.ap(), h_sl[:])

    in_maps = []
    for c in range(NCORES):
        in_maps.append({
            "wstream": wstream[c], "masks": maskbuf[c],
            "xt": xT_b, "iouxstat": iouxstat[c], "iouhstat": iouhstat[c],
            "fxstat": fxstat[c], "fhstat": fhstat[c],
            "b_xi": b_xi[c], "b_iou": b_iou[c], "b_xf": b_xf[c],
            "b_fh": b_fh[c],
        })
    _split_multi_waits(nc)
    kernel._nc = nc
    kernel._in_maps = in_maps
    res = run_bass_kernel_spmd(nc, in_maps, list(range(NCORES)))
    hT = np.concatenate([res.results[c]["hout"] for c in range(NCORES)], 0)
    out = np.empty((N, MEM), np.float32)
    for node in range(N):
        out[node] = hT[:, col_of[node]]
    return out
